# revision 29
# baseline (speedup 1.0000x reference)
"""TGCN (AttentionGNN) distributed Bass kernel for 8 TRN2 NeuronCores.

Math restructuring vs reference:
  gcn(xt, W, b) = (A_norm @ xt) @ W + b   (aggregation commutes with the
  feature transform), so we aggregate RAW features once:
      Xagg = A_norm @ X          X: [N, 192]  (192 = 16 feats x 12 steps)
  and fold the GCN weights into the GRU input transforms on the host:
      WgL = Wg @ LgW[:32],  bg2 = bg @ LgW[:32] + Lgb,  Ug = LgW[32:]
  Per step:  Z = sig(Xagg_t @ WzL + H @ Uz + bz2)  etc.
  The GRU is per-node independent -> zero cross-core communication after
  node partitioning.  Each core returns relu(Hacc).sum(nodes) [4x32];
  the host finishes mean + final linear.

Device phase 1 (per core): dma_gather edge source rows from a replicated
fp8-e3m4 feature table (256B rows), scatter-add into per-block PSUM
accumulators via one-hot matmuls (edges grouped by dst block on host),
PE-transpose blocks into [features, nodes] layout.  Gathers use 2048-idx
calls (single_packet=False) spread over the 4 SWDGE queues; the SWDGE
descriptor-emission rate (~2.2 ns/row across 4 queues) is the roofline.

The GRU scan is interleaved INTO phase 1: nodes are packed as 4 quarters
of 32 hidden rows (128 partitions), columns split into 4 groups; as soon
as a group's blocks are aggregated, its 12 GRU steps are emitted one per
subsequent block so the scan hides under the gather stream.
"""

import sys

if '/opt/trn_rl_repo' not in sys.path:
    sys.path.insert(0, '/opt/trn_rl_repo')

import heapq
import os
from contextlib import ExitStack
from dataclasses import dataclass

import ml_dtypes
import numpy as np

import concourse.bacc as bacc
import concourse.mybir as mybir
import concourse.tile as tile
from concourse.bass_utils import run_bass_kernel_spmd
from concourse.library_config import mlp

F32 = mybir.dt.float32
BF16 = mybir.dt.bfloat16
FP8 = mybir.dt.float8e3
I16 = mybir.dt.int16
AF = mybir.ActivationFunctionType
ALU = mybir.AluOpType

TAB_SCALE = 2.0           # global pow2 scale into fp8 e3m4 (max |x*dis| ~4.8)


def cdiv(a, b):
    return -(-a // b)


@dataclass
class Cfg:
    n: int = 50000          # nodes
    f: int = 16             # input feats
    t: int = 12             # time steps
    hid: int = 32
    ncores: int = 8
    nb: int = 49            # dst blocks per core
    qb: int = 13            # block columns per quarter (4 quarters)
    cha: int = 17           # chunks (of 128 edges) per block, table half A
    chb: int = 15           # chunks per block, table half B
    slice_a: tuple = (0, 32768)
    slice_b: tuple = (17232, 50000)
    gidx: int = 2048        # indices per dma_gather (single_packet=False)

    @property
    def npc(self):          # real nodes per core
        return self.n // self.ncores

    @property
    def cpb(self):          # chunks per block
        return self.cha + self.chb

    def ngather(self, half):
        return cdiv(self.nb * (self.cha, self.chb)[half] * 128, self.gidx)

    @property
    def fd(self):           # flattened feature dim
        return self.f * self.t

    @property
    def groups(self):       # j-columns per GRU group
        return [[8, 9], [10, 11], [12], [0, 1, 2, 3], [4, 5, 6, 7]]

    @property
    def porder(self):       # processing order: (quarter, j) pairs
        return [(q, j) for grp in self.groups for j in grp
                for q in range(4) if q * self.qb + j < self.nb]

    @property
    def ngroups(self):
        return len(self.groups)


def wrap_idx(idx1d):
    """[n] -> [128, n/16] dma_gather layout: index i at [i%16, i//16], x8."""
    n = idx1d.shape[0]
    assert n % 16 == 0
    return np.tile(idx1d.reshape(n // 16, 16).T, (8, 1)).astype(np.int16)


def partition_graph(cfg, edge_index):
    """Host-side graph partition. Returns per-core edge stream arrays."""
    N, NC, NB, CPB = cfg.n, cfg.ncores, cfg.nb, cfg.cpb
    src0 = np.asarray(edge_index[0], dtype=np.int64)
    dst0 = np.asarray(edge_index[1], dtype=np.int64)
    deg = np.bincount(dst0, minlength=N).astype(np.int64) + 1
    dis = (1.0 / np.sqrt(deg)).astype(np.float32)

    # Self loops are NOT in the gather stream: each block's own rows are
    # bulk-loaded (selftab) and scatter-added with an identity one-hot.
    # dis factors fold into the table (src) and per-block psum scale (dst).
    src, dst = src0, dst0

    # ---- assign nodes to (core, block, pos): greedy balance by in-degree
    nbuckets = NC * NB
    caps = np.full(nbuckets, 128, dtype=np.int64)
    leftover = cfg.npc - (NB - 1) * 128      # nodes in last block of a core
    assert 0 < leftover <= 128
    for c in range(NC):
        caps[c * NB + NB - 1] = leftover
    order = np.argsort(-deg, kind='stable')
    heap = [(0, b) for b in range(nbuckets)]
    heapq.heapify(heap)
    counts = np.zeros(nbuckets, dtype=np.int64)
    sums = np.zeros(nbuckets, dtype=np.int64)
    node_bucket = np.empty(N, dtype=np.int64)
    node_pos = np.empty(N, dtype=np.int64)
    for nidx in order:
        while True:
            s, b = heapq.heappop(heap)
            if counts[b] < caps[b] and s == sums[b]:
                break
        node_bucket[nidx] = b
        node_pos[nidx] = counts[b]
        counts[b] += 1
        sums[b] += deg[nidx]
        if counts[b] < caps[b]:
            heapq.heappush(heap, (sums[b], b))
    assert (counts == caps).all()

    slot_ids = np.full((nbuckets, 128), N, dtype=np.int64)   # N -> empty
    slot_ids[node_bucket, node_pos] = np.arange(N)

    eb = node_bucket[dst]                     # bucket of each edge
    epos = node_pos[dst]                      # slot-in-block of each edge
    # dis of the node occupying each (bucket, pos); 0 for empty slots
    disdst_all = np.zeros((NC * NB, 128), np.float32)
    disdst_all[node_bucket, node_pos] = dis
    disdst_all = disdst_all / TAB_SCALE       # undo the table's fp8 scale

    lo_a, hi_a = cfg.slice_a
    lo_b, hi_b = cfg.slice_b
    halfA_cap = cfg.cha * 128
    halfB_cap = cfg.chb * 128
    porder_b = [q * cfg.qb + j for q, j in cfg.porder]

    per_core = []
    for c in range(NC):
        idx_streams = {0: [], 1: []}
        dstloc_cols = []
        for blk in range(NB):
            b = c * NB + blk
            sel = np.nonzero(eb == b)[0]
            es, ep = src[sel], epos[sel]
            ne = es.shape[0]
            assert ne <= CPB * 128, f"block overflow {ne} > {CPB*128}"
            strictA = es < lo_b
            strictB = es >= hi_a
            ovl = ~strictA & ~strictB
            nA0 = int(strictA.sum())
            novl = int(ovl.sum())
            assert nA0 <= halfA_cap, f"strictA overflow {nA0}"
            # fill A from the overlap so that B fits its cap
            need = max(0, (ne - nA0 - novl) + novl - halfB_cap)
            fill = min(novl, max(need, 0))
            fill = max(fill, 0)
            assert nA0 + fill <= halfA_cap, f"A overflow {nA0}+{fill}"
            ovl_idx = np.nonzero(ovl)[0]
            inA = strictA.copy()
            inA[ovl_idx[:fill]] = True
            nB = ne - int(inA.sum())
            assert nB <= halfB_cap, f"B overflow {nB}"

            def padded(mask, base, cap):
                e_i = es[mask]
                p_i = ep[mask]
                pad = cap - e_i.shape[0]
                idxv = np.concatenate([e_i - base, np.zeros(pad, np.int64)])
                dl = np.concatenate([p_i, np.full(pad, -1.0)])
                return idxv, dl.astype(np.float32)

            iA, dlA = padded(inA, lo_a, halfA_cap)
            iB, dlB = padded(~inA, lo_b, halfB_cap)
            assert iA.max(initial=0) < hi_a - lo_a
            assert iB.max(initial=0) < hi_b - lo_b
            idx_streams[0].append(iA)
            idx_streams[1].append(iB)
            dstloc_cols.append(np.concatenate([dlA, dlB]))

        arrs = {}
        idx_streams = {t: [idx_streams[t][b] for b in porder_b]
                       for t in idx_streams}
        dstloc_cols = [dstloc_cols[b] for b in porder_b]
        for s in (0, 1):
            ng = cfg.ngather(s)
            st = np.concatenate(idx_streams[s])
            st = np.concatenate(
                [st, np.zeros(ng * cfg.gidx - st.shape[0], np.int64)])
            w = np.zeros((128, ng * (cfg.gidx // 16)), np.int16)
            iw = cfg.gidx // 16
            for g in range(ng):
                w[:, g * iw:(g + 1) * iw] = wrap_idx(
                    st[g * cfg.gidx:(g + 1) * cfg.gidx])
            arrs['idxA' if s == 0 else 'idxB'] = w
        # [128, NB*CPB]: chunk col-major; edge i of chunk at partition i%128
        dl = np.stack(dstloc_cols).reshape(NB * CPB, 128).T
        arrs['dstloc'] = dl.astype(ml_dtypes.bfloat16)
        arrs['disdst'] = disdst_all[c * NB:(c + 1) * NB][porder_b].T.copy()
        arrs['selfids'] = slot_ids[c * NB:(c + 1) * NB][porder_b]   # [NB,128]
        per_core.append(arrs)
    return per_core


def fold_weights(inp):
    HID = inp['LzW'].shape[1]
    out = {}
    wl = [np.asarray(inp[f'W{g}'], np.float32) @ np.asarray(inp[f'L{g}W'], np.float32)[:HID]
          for g in 'zrh']
    wf = np.concatenate(wl, axis=1)
    F = wf.shape[0]

    def bd4(m):
        o = np.zeros((128, 128), np.float32)
        for k in range(4):
            o[32 * k:32 * k + m.shape[0], 32 * k:32 * k + m.shape[1]] = m
        return o

    # X-side: per (parity, gate) [32,32] block (real rows at par*F), x4
    wxbd = np.zeros((128, 6 * 128), np.float32)
    for par in (0, 1):
        for g in range(3):
            blk = np.zeros((32, 32), np.float32)
            blk[par * F:(par + 1) * F] = wf[:, 32 * g:32 * g + 32]
            wxbd[:, (par * 3 + g) * 128:(par * 3 + g + 1) * 128] = bd4(blk)
    out['wxbd'] = wxbd.astype(ml_dtypes.bfloat16)
    uz = np.asarray(inp['LzW'], np.float32)[HID:]
    ur = np.asarray(inp['LrW'], np.float32)[HID:]
    uhm = np.asarray(inp['LhW'], np.float32)[HID:]
    out['ubd'] = np.concatenate([bd4(uz), bd4(ur), bd4(uhm)],
                                axis=1).astype(ml_dtypes.bfloat16)
    bl = [np.asarray(inp[f'b{g}'], np.float32) @ np.asarray(inp[f'L{g}W'], np.float32)[:HID]
          + np.asarray(inp[f'L{g}b'], np.float32) for g in 'zrh']
    out['bias'] = np.tile(np.stack(bl, axis=1), (4, 1)).astype(np.float32)
    att = np.asarray(inp['att'], np.float32)
    e = np.exp(att - att.max())
    out['probs'] = (e / e.sum()).astype(np.float32)
    return out


def make_table(cfg, x, edge_index):
    dst0 = np.asarray(edge_index[1], dtype=np.int64)
    deg = np.bincount(dst0, minlength=cfg.n).astype(np.int64) + 1
    dis = (1.0 / np.sqrt(deg)).astype(np.float32)
    xt = np.asarray(x, np.float32).transpose(0, 2, 1).reshape(cfg.n, cfg.fd)
    xt = xt * (dis[:, None] * TAB_SCALE)
    tab = np.zeros((cfg.n, 256), ml_dtypes.float8_e3m4)
    tab[:, :cfg.fd] = xt.astype(ml_dtypes.float8_e3m4)
    return tab


def build_nc(cfg, probs, bias0=False):
    NB, CPB, QB = cfg.nb, cfg.cpb, cfg.qb
    CHA, CHB = cfg.cha, cfg.chb
    NGA, NGB = cfg.ngather(0), cfg.ngather(1)
    GI = cfg.gidx
    IW = GI // 16
    CPG = GI // 128                       # chunks per gather
    NPAIR = cfg.t // 2
    GROUPS = cfg.groups
    PORDER = cfg.porder
    GW = [len(grp) * 128 for grp in GROUPS]
    GBUFS = int(os.environ.get('TGCN_GBUFS', '5'))

    nc = bacc.Bacc("TRN2", target_bir_lowering=False, debug=False,
                   num_devices=cfg.ncores, num_swdge_queues=4)
    xtab = nc.dram_tensor("xtab", [cfg.n, 256], FP8, kind="ExternalInput")
    idxA = nc.dram_tensor("idxA", [128, NGA * IW], I16, kind="ExternalInput")
    idxB = nc.dram_tensor("idxB", [128, NGB * IW], I16, kind="ExternalInput")
    dstloc = nc.dram_tensor("dstloc", [128, NB * CPB], BF16, kind="ExternalInput")
    disdst = nc.dram_tensor("disdst", [128, NB], F32, kind="ExternalInput")
    selftab = nc.dram_tensor("selftab", [128, NB * 256], FP8, kind="ExternalInput")
    wxbd = nc.dram_tensor("wxbd", [128, 6 * 128], BF16, kind="ExternalInput")
    ubd = nc.dram_tensor("ubd", [128, 384], BF16, kind="ExternalInput")
    bias = nc.dram_tensor("bias", [128, 3], F32, kind="ExternalInput")
    identw = nc.dram_tensor("identw", [128, 128], BF16, kind="ExternalInput")
    iotaw = nc.dram_tensor("iotaw", [128, 128], BF16, kind="ExternalInput")
    out = nc.dram_tensor("out", [128, len(GROUPS)], F32, kind="ExternalOutput")

    with tile.TileContext(nc) as tc, ExitStack() as ctx:
        cpool = ctx.enter_context(tc.tile_pool(name="const", bufs=1))
        gpool = ctx.enter_context(tc.tile_pool(name="gath", bufs=GBUFS))
        opool = ctx.enter_context(tc.tile_pool(name="oh", bufs=2))
        tpool = ctx.enter_context(tc.tile_pool(name="xbp", bufs=2))
        ppool = ctx.enter_context(tc.tile_pool(name="pb", bufs=1, space="PSUM"))
        ptpool = ctx.enter_context(tc.tile_pool(name="pt", bufs=1, space="PSUM"))
        psum2 = ctx.enter_context(tc.tile_pool(name="ps2", bufs=1, space="PSUM"))
        p2pool = ctx.enter_context(tc.tile_pool(name="p2", bufs=3))

        nc.gpsimd.load_library(mlp)

        idxA_sb = cpool.tile([128, NGA * IW], I16)
        idxB_sb = cpool.tile([128, NGB * IW], I16)
        for lo, hi in ((0, 8), (8, NGA)):
            nc.sync.dma_start(idxA_sb[:, lo * IW:hi * IW],
                              idxA[:, lo * IW:hi * IW])
        for lo, hi in ((0, 8), (8, NGB)):
            nc.sync.dma_start(idxB_sb[:, lo * IW:hi * IW],
                              idxB[:, lo * IW:hi * IW])
        dstloc_sb = cpool.tile([128, NB * CPB], BF16)
        nc.sync.dma_start(dstloc_sb[:], dstloc[:])
        disdst_sb = cpool.tile([128, NB], F32)
        nc.sync.dma_start(disdst_sb[:], disdst[:])
        self_sb = cpool.tile([128, NB, 256], FP8)
        nc.sync.dma_start(self_sb[:], selftab[:].rearrange("p (b e) -> p b e", e=256))
        wxbd_sb = cpool.tile([128, 6 * 128], BF16)
        nc.sync.dma_start(wxbd_sb[:], wxbd[:])
        ubd_sb = cpool.tile([128, 384], BF16)
        nc.sync.dma_start(ubd_sb[:], ubd[:])
        bias_sb = cpool.tile([128, 3], F32)
        nc.sync.dma_start(bias_sb[:], bias[:])

        ident = cpool.tile([128, 128], BF16)
        nc.sync.dma_start(ident[:], identw[:])
        iota128 = cpool.tile([128, 128], BF16)
        nc.sync.dma_start(iota128[:], iotaw[:])

        xp_g = [cpool.tile([128, NPAIR * gw], BF16, name=f"xp{i}")
                for i, gw in enumerate(GW)]
        H_g = [cpool.tile([128, gw], BF16, name=f"H{i}")
               for i, gw in enumerate(GW)]
        acc_g = [cpool.tile([128, gw], F32, name=f"acc{i}")
                 for i, gw in enumerate(GW)]
        out_sb = cpool.tile([128, len(GROUPS)], F32)
        for t_ in H_g + acc_g:
            nc.vector.memset(t_[:], 0.0)
        for gi, grp in enumerate(GROUPS):
            if any(3 * cfg.qb + j >= NB for j in grp):
                nc.vector.memset(xp_g[gi][96:128, :], 0.0)

        # ---------------- gather stream ----------------
        slA = xtab[cfg.slice_a[0]:cfg.slice_a[1], :]
        slB = xtab[cfg.slice_b[0]:cfg.slice_b[1], :]
        gtiles = {}
        schunks = {0: NB * CHA, 1: NB * CHB}
        qcnt = [0]

        def gather_tile(s, g):
            if (s, g) not in gtiles:
                rem = min(CPG, schunks[s] - g * CPG)
                ni = rem * 128
                t = gpool.tile([128, CPG, 256], FP8, tag=f"g{s}")
                isb = idxA_sb if s == 0 else idxB_sb
                nc.gpsimd.dma_gather(
                    t[:, :rem, :], slA if s == 0 else slB,
                    isb[:, g * IW:g * IW + ni // 16], ni, ni, 256,
                    queue_num=qcnt[0] % 4, single_packet=False)
                qcnt[0] += 1
                gtiles[(s, g)] = t
            return gtiles[(s, g)]

        # ---------------- GRU step ----------------
        def gru_step(g, t, tail=False):
            gw = GW[g]
            ts = 'AB'[g % 2]
            pair, par = divmod(t, 2)
            pt_f = float(probs[t])
            H = H_g[g]
            pszr = psum2.tile([128, 1024], F32, tag=f"pszr{ts}")
            psh = psum2.tile([128, 512], F32, tag=f"psh{ts}")
            psz = pszr[:, 0:512]
            psr = pszr[:, 512:1024]
            xrow = xp_g[g][:, pair * gw:(pair + 1) * gw]
            wb = par * 3 * 128
            nc.tensor.matmul(psz[:, :gw], lhsT=wxbd_sb[:, wb:wb + 128],
                             rhs=xrow, start=True, stop=False)
            nc.tensor.matmul(psz[:, :gw], lhsT=ubd_sb[:, 0:128], rhs=H[:],
                             start=False, stop=True)
            nc.tensor.matmul(psr[:, :gw], lhsT=wxbd_sb[:, wb + 128:wb + 256],
                             rhs=xrow, start=True, stop=False)
            nc.tensor.matmul(psr[:, :gw], lhsT=ubd_sb[:, 128:256], rhs=H[:],
                             start=False, stop=True)
            nc.tensor.matmul(psh[:, :gw], lhsT=wxbd_sb[:, wb + 256:wb + 384],
                             rhs=xrow, start=True, stop=False)
            zrt = p2pool.tile([128, 1024], BF16, tag=f"zrt{ts}")
            zt = zrt[:, 0:512]
            rt = zrt[:, 512:1024]
            if bias0 and gw == 512:
                nc.scalar.activation(zrt[:], pszr[:], AF.Sigmoid)
            elif bias0:
                nc.scalar.activation(zt[:, :gw], psz[:, :gw], AF.Sigmoid)
                nc.scalar.activation(rt[:, :gw], psr[:, :gw], AF.Sigmoid)
            else:
                nc.scalar.activation(zt[:, :gw], psz[:, :gw], AF.Sigmoid,
                                     bias=bias_sb[:, 0:1])
                nc.scalar.activation(rt[:, :gw], psr[:, :gw], AF.Sigmoid,
                                     bias=bias_sb[:, 1:2])
            rh = p2pool.tile([128, 512], BF16, tag=f"rh{ts}")
            nc.vector.tensor_tensor(rh[:, :gw], rt[:, :gw], H[:], op=ALU.mult)
            nc.tensor.matmul(psh[:, :gw], lhsT=ubd_sb[:, 256:384],
                             rhs=rh[:, :gw], start=False, stop=True)
            ht = p2pool.tile([128, 512], BF16, tag=f"ht{ts}")
            if bias0:
                nc.scalar.activation(ht[:, :gw], psh[:, :gw], AF.Tanh)
            else:
                nc.scalar.activation(ht[:, :gw], psh[:, :gw], AF.Tanh,
                                     bias=bias_sb[:, 2:3])
            t1 = p2pool.tile([128, 512], BF16, tag=f"t1{ts}")
            nc.vector.tensor_sub(t1[:, :gw], H[:], ht[:, :gw])
            nc.vector.tensor_tensor(t1[:, :gw], zt[:, :gw], t1[:, :gw],
                                    op=ALU.mult)
            nc.vector.tensor_add(H[:], t1[:, :gw], ht[:, :gw])
            nc.vector.scalar_tensor_tensor(
                acc_g[g][:], H[:], pt_f, acc_g[g][:], op0=ALU.mult, op1=ALU.add)

        # quarter 3 holds blocks j=0..NB-1-3*QB; its last block is partial.
        leftover = cfg.npc - (NB - 1) * 128
        q3_valid = (NB - 1 - 3 * QB) * 128 + leftover

        def readout(g):
            acc = acc_g[g]
            nc.scalar.activation(acc[:], acc[:], AF.Relu)
            # valid columns: quarters 0-2 full; quarter 3 (rows 96:128) only
            # has real nodes up to per-quarter col q3_valid.
            glo = min(GROUPS[g]) * 128           # group's per-quarter col start
            v3 = min(max(q3_valid - glo, 0), GW[g])
            if v3 == GW[g]:
                nc.vector.tensor_reduce(out_sb[:, g:g + 1], acc[:],
                                        axis=mybir.AxisListType.X, op=ALU.add)
            else:
                nc.vector.tensor_reduce(out_sb[0:96, g:g + 1], acc[0:96, :],
                                        axis=mybir.AxisListType.X, op=ALU.add)
                if v3 > 0:
                    nc.vector.tensor_reduce(out_sb[96:128, g:g + 1],
                                            acc[96:128, :v3],
                                            axis=mybir.AxisListType.X,
                                            op=ALU.add)
                else:
                    nc.vector.memset(out_sb[96:128, g:g + 1], 0.0)

        # ---------------- phase 1 + interleaved GRU ----------------
        ngrp = len(GROUPS)
        jgrp = {j: gi for gi, grp in enumerate(GROUPS) for j in grp}
        done_at = {}                        # group -> #blocks done when ready
        seen = [0] * ngrp
        for q, j in PORDER:
            seen[jgrp[j]] += 1
        cum = 0
        for g in range(ngrp):
            cum += seen[g]
            done_at[g] = cum
        pending = [(g, t) for g in range(ngrp) for t in range(cfg.t)]
        emitted = 0

        for bi, (q, j) in enumerate(PORDER):
            g = jgrp[j]
            lj = j - min(GROUPS[g])
            psumb = ppool.tile([128, cfg.fd], F32, tag="pb")
            oc = opool.tile([128, CPB, 128], BF16, tag="oc")
            nc.vector.tensor_tensor(
                oc[:],
                dstloc_sb[:, bi * CPB:(bi + 1) * CPB]
                .rearrange("p (c o) -> p c o", o=1).to_broadcast([128, CPB, 128]),
                iota128[:].rearrange("p (o s) -> p o s", o=1).to_broadcast([128, CPB, 128]),
                op=ALU.is_equal)
            for jj in range(CPB):
                s = 0 if jj < CHA else 1
                c = bi * CHA + jj if s == 0 else bi * CHB + (jj - CHA)
                gg, slot = divmod(c, CPG)
                gt = gather_tile(s, gg)
                nc.tensor.matmul(
                    psumb[:], lhsT=oc[:, jj, :], rhs=gt[:, slot, :cfg.fd],
                    start=(jj == 0), stop=False)
            nc.tensor.matmul(psumb[:], lhsT=ident[:],
                             rhs=self_sb[:, bi, :cfg.fd],
                             start=False, stop=True)
            xbp = tpool.tile([128, NPAIR, 128], BF16, tag="xb")
            nc.scalar.activation(
                xbp[:, :, 32 * q:32 * q + 32],
                psumb[:].rearrange("p (a b) -> p a b", a=NPAIR),
                AF.Copy, scale=disdst_sb[:, bi:bi + 1])
            for p in range(NPAIR):
                pt = ptpool.tile([128, 128], BF16, tag="pt")
                nc.tensor.transpose(pt[:], xbp[:, p, :], ident[:])
                nc.scalar.copy(
                    xp_g[g][32 * q:32 * q + 32,
                            p * GW[g] + lj * 128:p * GW[g] + (lj + 1) * 128],
                    pt[32 * q:32 * q + 32, :])
            # one pending GRU step per completed block
            if pending and done_at[pending[0][0]] <= bi + 1:
                sg, st = pending.pop(0)
                gru_step(sg, st)
                if st == cfg.t - 1:
                    readout(sg)

        # ---------------- tail: remaining GRU steps ----------------
        rem = {}
        for g, t in pending:
            rem.setdefault(g, []).append(t)
        while rem:
            for g in sorted(rem):
                t = rem[g].pop(0)
                gru_step(g, t, tail=True)
                if t == cfg.t - 1:
                    readout(g)
                if not rem[g]:
                    del rem[g]

        nc.sync.dma_start(out[:], out_sb[:])

    # Spread gathers over the 4 SWDGE queues in final instruction order so
    # the DMASW semaphore-lane rotation maps each lane to one queue.
    cnt = 0
    for f in nc.m.functions:
        for bb in f.blocks:
            for ins in bb.instructions:
                if isinstance(ins, mybir.InstDMAGatherAnt):
                    ins.queue_num = (cnt % 8) % 4
                    cnt += 1

    nc.compile()
    return nc


def _run(cfg=None, trace=False, **inputs):
    if cfg is None:
        cfg = Cfg()
    per_core = partition_graph(cfg, np.asarray(inputs['edge_index']))
    folded = fold_weights(inputs)
    xtab = make_table(cfg, inputs['x'], np.asarray(inputs['edge_index']))
    tab_pad = np.vstack([xtab, np.zeros((1, 256), xtab.dtype)])
    for pc in per_core:
        ids = pc.pop('selfids')                       # [NB, 128]
        st = tab_pad[ids]                             # [NB, 128, 256]
        pc['selftab'] = np.ascontiguousarray(
            st.transpose(1, 0, 2).reshape(128, -1))
    nc = build_nc(cfg, folded['probs'], bias0=not np.any(folded['bias']))
    identw = np.eye(128, dtype=ml_dtypes.bfloat16)
    iotaw = np.tile(np.arange(128, dtype=ml_dtypes.bfloat16), (128, 1))
    shared = {'xtab': xtab, 'wxbd': folded['wxbd'], 'ubd': folded['ubd'],
              'bias': folded['bias'], 'identw': identw, 'iotaw': iotaw}
    in_maps = [{**shared, **pc} for pc in per_core]
    res = run_bass_kernel_spmd(nc, in_maps, core_ids=list(range(cfg.ncores)),
                               trace=trace)
    hsum = np.zeros(cfg.hid, np.float64)
    for r in res.results:
        hsum += (r['out'].astype(np.float64).sum(axis=1)
                 .reshape(4, cfg.hid).sum(0))
    hbar = (hsum / cfg.n).astype(np.float32)[None, :]
    linW = np.asarray(inputs['linW'], np.float32)
    linb = np.asarray(inputs['linb'], np.float32)
    y = np.maximum(hbar @ linW + linb, 0.0).astype(np.float32)
    return y, res


def kernel(**inputs):
    """Grading entry point: full inputs in, full [1, 1] output back."""
    y, _res = _run(cfg=None, trace=False, **inputs)
    return y


# revision 30
# speedup vs baseline: 1.0151x; 1.0151x over previous
"""TGCN (AttentionGNN) distributed Bass kernel for 8 TRN2 NeuronCores.

Math restructuring vs reference:
  gcn(xt, W, b) = (A_norm @ xt) @ W + b   (aggregation commutes with the
  feature transform), so we aggregate RAW features once:
      Xagg = A_norm @ X          X: [N, 192]  (192 = 16 feats x 12 steps)
  and fold the GCN weights into the GRU input transforms on the host:
      WgL = Wg @ LgW[:32],  bg2 = bg @ LgW[:32] + Lgb,  Ug = LgW[32:]
  Per step:  Z = sig(Xagg_t @ WzL + H @ Uz + bz2)  etc.
  The GRU is per-node independent -> zero cross-core communication after
  node partitioning.  Each core returns relu(Hacc).sum(nodes) [4x32];
  the host finishes mean + final linear.

Device phase 1 (per core): dma_gather edge source rows from a replicated
fp8-e3m4 feature table (256B rows), scatter-add into per-block PSUM
accumulators via one-hot matmuls (edges grouped by dst block on host),
PE-transpose blocks into [features, nodes] layout.  Gathers use 2048-idx
calls (single_packet=False) spread over the 4 SWDGE queues; the SWDGE
descriptor-emission rate (~2.2 ns/row across 4 queues) is the roofline.

The GRU scan is interleaved INTO phase 1: nodes are packed as 4 quarters
of 32 hidden rows (128 partitions), columns split into 4 groups; as soon
as a group's blocks are aggregated, its 12 GRU steps are emitted one per
subsequent block so the scan hides under the gather stream.
"""

import sys

if '/opt/trn_rl_repo' not in sys.path:
    sys.path.insert(0, '/opt/trn_rl_repo')

import heapq
import os
from contextlib import ExitStack
from dataclasses import dataclass

import ml_dtypes
import numpy as np

import concourse.bacc as bacc
import concourse.mybir as mybir
import concourse.tile as tile
from concourse.bass_utils import run_bass_kernel_spmd
from concourse.library_config import mlp

F32 = mybir.dt.float32
BF16 = mybir.dt.bfloat16
FP8 = mybir.dt.float8e3
I16 = mybir.dt.int16
AF = mybir.ActivationFunctionType
ALU = mybir.AluOpType

TAB_SCALE = 2.0           # global pow2 scale into fp8 e3m4 (max |x*dis| ~4.8)


def cdiv(a, b):
    return -(-a // b)


@dataclass
class Cfg:
    n: int = 50000          # nodes
    f: int = 16             # input feats
    t: int = 12             # time steps
    hid: int = 32
    ncores: int = 8
    nb: int = 49            # dst blocks per core
    qb: int = 13            # block columns per quarter (4 quarters)
    cha: int = 17           # chunks (of 128 edges) per block, table half A
    chb: int = 15           # chunks per block, table half B
    slice_a: tuple = (0, 32768)
    slice_b: tuple = (17232, 50000)
    gidx: int = 2048        # indices per dma_gather (single_packet=False)

    @property
    def npc(self):          # real nodes per core
        return self.n // self.ncores

    @property
    def cpb(self):          # chunks per block
        return self.cha + self.chb

    def ngather(self, half):
        return cdiv(self.nb * (self.cha, self.chb)[half] * 128, self.gidx)

    @property
    def fd(self):           # flattened feature dim
        return self.f * self.t

    @property
    def groups(self):       # j-columns per GRU group
        return [[8, 9], [10, 11], [12], [0, 1, 2, 3], [4, 5, 6, 7]]

    @property
    def porder(self):       # processing order: (quarter, j) pairs
        return [(q, j) for grp in self.groups for j in grp
                for q in range(4) if q * self.qb + j < self.nb]

    @property
    def ngroups(self):
        return len(self.groups)


def wrap_idx(idx1d):
    """[n] -> [128, n/16] dma_gather layout: index i at [i%16, i//16], x8."""
    n = idx1d.shape[0]
    assert n % 16 == 0
    return np.tile(idx1d.reshape(n // 16, 16).T, (8, 1)).astype(np.int16)


def partition_graph(cfg, edge_index):
    """Host-side graph partition. Returns per-core edge stream arrays."""
    N, NC, NB, CPB = cfg.n, cfg.ncores, cfg.nb, cfg.cpb
    src0 = np.asarray(edge_index[0], dtype=np.int64)
    dst0 = np.asarray(edge_index[1], dtype=np.int64)
    deg = np.bincount(dst0, minlength=N).astype(np.int64) + 1
    dis = (1.0 / np.sqrt(deg)).astype(np.float32)

    # Self loops are NOT in the gather stream: each block's own rows are
    # bulk-loaded (selftab) and scatter-added with an identity one-hot.
    # dis factors fold into the table (src) and per-block psum scale (dst).
    src, dst = src0, dst0

    # ---- assign nodes to (core, block, pos): greedy balance by in-degree
    nbuckets = NC * NB
    caps = np.full(nbuckets, 128, dtype=np.int64)
    leftover = cfg.npc - (NB - 1) * 128      # nodes in last block of a core
    assert 0 < leftover <= 128
    for c in range(NC):
        caps[c * NB + NB - 1] = leftover
    order = np.argsort(-deg, kind='stable')
    heap = [(0, b) for b in range(nbuckets)]
    heapq.heapify(heap)
    counts = np.zeros(nbuckets, dtype=np.int64)
    sums = np.zeros(nbuckets, dtype=np.int64)
    node_bucket = np.empty(N, dtype=np.int64)
    node_pos = np.empty(N, dtype=np.int64)
    for nidx in order:
        while True:
            s, b = heapq.heappop(heap)
            if counts[b] < caps[b] and s == sums[b]:
                break
        node_bucket[nidx] = b
        node_pos[nidx] = counts[b]
        counts[b] += 1
        sums[b] += deg[nidx]
        if counts[b] < caps[b]:
            heapq.heappush(heap, (sums[b], b))
    assert (counts == caps).all()

    slot_ids = np.full((nbuckets, 128), N, dtype=np.int64)   # N -> empty
    slot_ids[node_bucket, node_pos] = np.arange(N)

    eb = node_bucket[dst]                     # bucket of each edge
    epos = node_pos[dst]                      # slot-in-block of each edge
    # dis of the node occupying each (bucket, pos); 0 for empty slots
    disdst_all = np.zeros((NC * NB, 128), np.float32)
    disdst_all[node_bucket, node_pos] = dis
    disdst_all = disdst_all / TAB_SCALE       # undo the table's fp8 scale

    lo_a, hi_a = cfg.slice_a
    lo_b, hi_b = cfg.slice_b
    halfA_cap = cfg.cha * 128
    halfB_cap = cfg.chb * 128
    porder_b = [q * cfg.qb + j for q, j in cfg.porder]

    per_core = []
    for c in range(NC):
        idx_streams = {0: [], 1: []}
        dstloc_cols = []
        for blk in range(NB):
            b = c * NB + blk
            sel = np.nonzero(eb == b)[0]
            es, ep = src[sel], epos[sel]
            ne = es.shape[0]
            assert ne <= CPB * 128, f"block overflow {ne} > {CPB*128}"
            strictA = es < lo_b
            strictB = es >= hi_a
            ovl = ~strictA & ~strictB
            nA0 = int(strictA.sum())
            novl = int(ovl.sum())
            assert nA0 <= halfA_cap, f"strictA overflow {nA0}"
            # fill A from the overlap so that B fits its cap
            need = max(0, (ne - nA0 - novl) + novl - halfB_cap)
            fill = min(novl, max(need, 0))
            fill = max(fill, 0)
            assert nA0 + fill <= halfA_cap, f"A overflow {nA0}+{fill}"
            ovl_idx = np.nonzero(ovl)[0]
            inA = strictA.copy()
            inA[ovl_idx[:fill]] = True
            nB = ne - int(inA.sum())
            assert nB <= halfB_cap, f"B overflow {nB}"

            def padded(mask, base, cap):
                e_i = es[mask]
                p_i = ep[mask]
                pad = cap - e_i.shape[0]
                idxv = np.concatenate([e_i - base, np.zeros(pad, np.int64)])
                dl = np.concatenate([p_i, np.full(pad, -1.0)])
                return idxv, dl.astype(np.float32)

            iA, dlA = padded(inA, lo_a, halfA_cap)
            iB, dlB = padded(~inA, lo_b, halfB_cap)
            assert iA.max(initial=0) < hi_a - lo_a
            assert iB.max(initial=0) < hi_b - lo_b
            idx_streams[0].append(iA)
            idx_streams[1].append(iB)
            dstloc_cols.append(np.concatenate([dlA, dlB]))

        arrs = {}
        idx_streams = {t: [idx_streams[t][b] for b in porder_b]
                       for t in idx_streams}
        dstloc_cols = [dstloc_cols[b] for b in porder_b]
        for s in (0, 1):
            ng = cfg.ngather(s)
            st = np.concatenate(idx_streams[s])
            st = np.concatenate(
                [st, np.zeros(ng * cfg.gidx - st.shape[0], np.int64)])
            w = np.zeros((128, ng * (cfg.gidx // 16)), np.int16)
            iw = cfg.gidx // 16
            for g in range(ng):
                w[:, g * iw:(g + 1) * iw] = wrap_idx(
                    st[g * cfg.gidx:(g + 1) * cfg.gidx])
            arrs['idxA' if s == 0 else 'idxB'] = w
        # [128, NB*CPB]: chunk col-major; edge i of chunk at partition i%128
        dl = np.stack(dstloc_cols).reshape(NB * CPB, 128).T
        arrs['dstloc'] = dl.astype(ml_dtypes.bfloat16)
        arrs['disdst'] = disdst_all[c * NB:(c + 1) * NB][porder_b].T.copy()
        arrs['selfids'] = slot_ids[c * NB:(c + 1) * NB][porder_b]   # [NB,128]
        per_core.append(arrs)
    return per_core


def fold_weights(inp):
    HID = inp['LzW'].shape[1]
    out = {}
    wl = [np.asarray(inp[f'W{g}'], np.float32) @ np.asarray(inp[f'L{g}W'], np.float32)[:HID]
          for g in 'zrh']
    wf = np.concatenate(wl, axis=1)
    F = wf.shape[0]

    def bd4(m):
        o = np.zeros((128, 128), np.float32)
        for k in range(4):
            o[32 * k:32 * k + m.shape[0], 32 * k:32 * k + m.shape[1]] = m
        return o

    # X-side: per (parity, gate) [32,32] block (real rows at par*F), x4
    wxbd = np.zeros((128, 6 * 128), np.float32)
    for par in (0, 1):
        for g in range(3):
            blk = np.zeros((32, 32), np.float32)
            blk[par * F:(par + 1) * F] = wf[:, 32 * g:32 * g + 32]
            wxbd[:, (par * 3 + g) * 128:(par * 3 + g + 1) * 128] = bd4(blk)
    out['wxbd'] = wxbd.astype(ml_dtypes.bfloat16)
    uz = np.asarray(inp['LzW'], np.float32)[HID:]
    ur = np.asarray(inp['LrW'], np.float32)[HID:]
    uhm = np.asarray(inp['LhW'], np.float32)[HID:]
    out['ubd'] = np.concatenate([bd4(uz), bd4(ur), bd4(uhm)],
                                axis=1).astype(ml_dtypes.bfloat16)
    bl = [np.asarray(inp[f'b{g}'], np.float32) @ np.asarray(inp[f'L{g}W'], np.float32)[:HID]
          + np.asarray(inp[f'L{g}b'], np.float32) for g in 'zrh']
    out['bias'] = np.tile(np.stack(bl, axis=1), (4, 1)).astype(np.float32)
    att = np.asarray(inp['att'], np.float32)
    e = np.exp(att - att.max())
    out['probs'] = (e / e.sum()).astype(np.float32)
    return out


def make_table(cfg, x, edge_index):
    dst0 = np.asarray(edge_index[1], dtype=np.int64)
    deg = np.bincount(dst0, minlength=cfg.n).astype(np.int64) + 1
    dis = (1.0 / np.sqrt(deg)).astype(np.float32)
    xt = np.asarray(x, np.float32).transpose(0, 2, 1).reshape(cfg.n, cfg.fd)
    xt = xt * (dis[:, None] * TAB_SCALE)
    tab = np.zeros((cfg.n, 256), ml_dtypes.float8_e3m4)
    tab[:, :cfg.fd] = xt.astype(ml_dtypes.float8_e3m4)
    return tab


def build_nc(cfg, probs, bias0=False):
    NB, CPB, QB = cfg.nb, cfg.cpb, cfg.qb
    CHA, CHB = cfg.cha, cfg.chb
    NGA, NGB = cfg.ngather(0), cfg.ngather(1)
    GI = cfg.gidx
    IW = GI // 16
    CPG = GI // 128                       # chunks per gather
    NPAIR = cfg.t // 2
    GROUPS = cfg.groups
    PORDER = cfg.porder
    GW = [len(grp) * 128 for grp in GROUPS]
    GBUFS = int(os.environ.get('TGCN_GBUFS', '5'))

    nc = bacc.Bacc("TRN2", target_bir_lowering=False, debug=False,
                   num_devices=cfg.ncores, num_swdge_queues=4)
    xtab = nc.dram_tensor("xtab", [cfg.n, 256], FP8, kind="ExternalInput")
    idxA = nc.dram_tensor("idxA", [128, NGA * IW], I16, kind="ExternalInput")
    idxB = nc.dram_tensor("idxB", [128, NGB * IW], I16, kind="ExternalInput")
    dstloc = nc.dram_tensor("dstloc", [128, NB * CPB], BF16, kind="ExternalInput")
    disdst = nc.dram_tensor("disdst", [128, NB], F32, kind="ExternalInput")
    selftab = nc.dram_tensor("selftab", [128, NB * 256], FP8, kind="ExternalInput")
    wxbd = nc.dram_tensor("wxbd", [128, 6 * 128], BF16, kind="ExternalInput")
    ubd = nc.dram_tensor("ubd", [128, 384], BF16, kind="ExternalInput")
    bias = nc.dram_tensor("bias", [128, 3], F32, kind="ExternalInput")
    identw = nc.dram_tensor("identw", [128, 128], BF16, kind="ExternalInput")
    iotaw = nc.dram_tensor("iotaw", [128, 128], BF16, kind="ExternalInput")
    out = nc.dram_tensor("out", [128, len(GROUPS)], F32, kind="ExternalOutput")

    with tile.TileContext(nc) as tc, ExitStack() as ctx:
        cpool = ctx.enter_context(tc.tile_pool(name="const", bufs=1))
        gpool = ctx.enter_context(tc.tile_pool(name="gath", bufs=GBUFS))
        opool = ctx.enter_context(tc.tile_pool(name="oh", bufs=2))
        tpool = ctx.enter_context(tc.tile_pool(name="xbp", bufs=2))
        ppool = ctx.enter_context(tc.tile_pool(name="pb", bufs=1, space="PSUM"))
        ptpool = ctx.enter_context(tc.tile_pool(name="pt", bufs=1, space="PSUM"))
        psum2 = ctx.enter_context(tc.tile_pool(name="ps2", bufs=1, space="PSUM"))
        p2pool = ctx.enter_context(tc.tile_pool(name="p2", bufs=2))

        nc.gpsimd.load_library(mlp)

        idxA_sb = cpool.tile([128, NGA * IW], I16)
        idxB_sb = cpool.tile([128, NGB * IW], I16)
        for lo, hi in ((0, 8), (8, NGA)):
            nc.sync.dma_start(idxA_sb[:, lo * IW:hi * IW],
                              idxA[:, lo * IW:hi * IW])
        for lo, hi in ((0, 8), (8, NGB)):
            nc.sync.dma_start(idxB_sb[:, lo * IW:hi * IW],
                              idxB[:, lo * IW:hi * IW])
        dstloc_sb = cpool.tile([128, NB * CPB], BF16)
        nc.sync.dma_start(dstloc_sb[:], dstloc[:])
        disdst_sb = cpool.tile([128, NB], F32)
        nc.sync.dma_start(disdst_sb[:], disdst[:])
        self_sb = cpool.tile([128, NB, 256], FP8)
        nc.sync.dma_start(self_sb[:], selftab[:].rearrange("p (b e) -> p b e", e=256))
        wxbd_sb = cpool.tile([128, 6 * 128], BF16)
        nc.sync.dma_start(wxbd_sb[:], wxbd[:])
        ubd_sb = cpool.tile([128, 384], BF16)
        nc.sync.dma_start(ubd_sb[:], ubd[:])
        bias_sb = cpool.tile([128, 3], F32)
        nc.sync.dma_start(bias_sb[:], bias[:])

        ident = cpool.tile([128, 128], BF16)
        nc.sync.dma_start(ident[:], identw[:])
        iota128 = cpool.tile([128, 128], BF16)
        nc.sync.dma_start(iota128[:], iotaw[:])

        xp_g = [cpool.tile([128, NPAIR * gw], BF16, name=f"xp{i}")
                for i, gw in enumerate(GW)]
        H_g = [cpool.tile([128, gw], BF16, name=f"H{i}")
               for i, gw in enumerate(GW)]
        acc_g = [cpool.tile([128, gw], F32, name=f"acc{i}")
                 for i, gw in enumerate(GW)]
        out_sb = cpool.tile([128, len(GROUPS)], F32)
        for t_ in H_g + acc_g:
            nc.vector.memset(t_[:], 0.0)
        for gi, grp in enumerate(GROUPS):
            if any(3 * cfg.qb + j >= NB for j in grp):
                nc.vector.memset(xp_g[gi][96:128, :], 0.0)

        # ---------------- gather stream ----------------
        slA = xtab[cfg.slice_a[0]:cfg.slice_a[1], :]
        slB = xtab[cfg.slice_b[0]:cfg.slice_b[1], :]
        gtiles = {}
        schunks = {0: NB * CHA, 1: NB * CHB}
        qcnt = [0]

        def gather_tile(s, g):
            if (s, g) not in gtiles:
                rem = min(CPG, schunks[s] - g * CPG)
                ni = rem * 128
                t = gpool.tile([128, CPG, 256], FP8, tag=f"g{s}")
                isb = idxA_sb if s == 0 else idxB_sb
                nc.gpsimd.dma_gather(
                    t[:, :rem, :], slA if s == 0 else slB,
                    isb[:, g * IW:g * IW + ni // 16], ni, ni, 256,
                    queue_num=qcnt[0] % 4, single_packet=False)
                qcnt[0] += 1
                gtiles[(s, g)] = t
            return gtiles[(s, g)]

        # ---------------- GRU step ----------------
        def gru_step(g, t, tail=False):
            gw = GW[g]
            ts = 'AB'[g % 2]
            pair, par = divmod(t, 2)
            pt_f = float(probs[t])
            H = H_g[g]
            pszr = psum2.tile([128, 1024], F32, tag=f"pszr{ts}")
            psh = psum2.tile([128, 512], F32, tag=f"psh{ts}")
            psz = pszr[:, 0:512]
            psr = pszr[:, 512:1024]
            xrow = xp_g[g][:, pair * gw:(pair + 1) * gw]
            wb = par * 3 * 128
            nc.tensor.matmul(psz[:, :gw], lhsT=wxbd_sb[:, wb:wb + 128],
                             rhs=xrow, start=True, stop=False)
            nc.tensor.matmul(psz[:, :gw], lhsT=ubd_sb[:, 0:128], rhs=H[:],
                             start=False, stop=True)
            nc.tensor.matmul(psr[:, :gw], lhsT=wxbd_sb[:, wb + 128:wb + 256],
                             rhs=xrow, start=True, stop=False)
            nc.tensor.matmul(psr[:, :gw], lhsT=ubd_sb[:, 128:256], rhs=H[:],
                             start=False, stop=True)
            nc.tensor.matmul(psh[:, :gw], lhsT=wxbd_sb[:, wb + 256:wb + 384],
                             rhs=xrow, start=True, stop=False)
            zrt = p2pool.tile([128, 1024], BF16, tag=f"zrt{ts}")
            zt = zrt[:, 0:512]
            rt = zrt[:, 512:1024]
            if bias0 and gw == 512:
                nc.scalar.activation(zrt[:], pszr[:], AF.Sigmoid)
            elif bias0:
                nc.scalar.activation(zt[:, :gw], psz[:, :gw], AF.Sigmoid)
                nc.scalar.activation(rt[:, :gw], psr[:, :gw], AF.Sigmoid)
            else:
                nc.scalar.activation(zt[:, :gw], psz[:, :gw], AF.Sigmoid,
                                     bias=bias_sb[:, 0:1])
                nc.scalar.activation(rt[:, :gw], psr[:, :gw], AF.Sigmoid,
                                     bias=bias_sb[:, 1:2])
            rh = p2pool.tile([128, 512], BF16, tag=f"rh{ts}")
            nc.vector.tensor_tensor(rh[:, :gw], rt[:, :gw], H[:], op=ALU.mult)
            nc.tensor.matmul(psh[:, :gw], lhsT=ubd_sb[:, 256:384],
                             rhs=rh[:, :gw], start=False, stop=True)
            ht = p2pool.tile([128, 512], BF16, tag=f"ht{ts}")
            if bias0:
                nc.scalar.activation(ht[:, :gw], psh[:, :gw], AF.Tanh)
            else:
                nc.scalar.activation(ht[:, :gw], psh[:, :gw], AF.Tanh,
                                     bias=bias_sb[:, 2:3])
            t1 = p2pool.tile([128, 512], BF16, tag=f"t1{ts}")
            nc.vector.tensor_sub(t1[:, :gw], H[:], ht[:, :gw])
            nc.vector.tensor_tensor(t1[:, :gw], zt[:, :gw], t1[:, :gw],
                                    op=ALU.mult)
            nc.vector.tensor_add(H[:], t1[:, :gw], ht[:, :gw])
            nc.vector.scalar_tensor_tensor(
                acc_g[g][:], H[:], pt_f, acc_g[g][:], op0=ALU.mult, op1=ALU.add)

        # quarter 3 holds blocks j=0..NB-1-3*QB; its last block is partial.
        leftover = cfg.npc - (NB - 1) * 128
        q3_valid = (NB - 1 - 3 * QB) * 128 + leftover

        def readout(g):
            acc = acc_g[g]
            nc.scalar.activation(acc[:], acc[:], AF.Relu)
            # valid columns: quarters 0-2 full; quarter 3 (rows 96:128) only
            # has real nodes up to per-quarter col q3_valid.
            glo = min(GROUPS[g]) * 128           # group's per-quarter col start
            v3 = min(max(q3_valid - glo, 0), GW[g])
            if v3 == GW[g]:
                nc.vector.tensor_reduce(out_sb[:, g:g + 1], acc[:],
                                        axis=mybir.AxisListType.X, op=ALU.add)
            else:
                nc.vector.tensor_reduce(out_sb[0:96, g:g + 1], acc[0:96, :],
                                        axis=mybir.AxisListType.X, op=ALU.add)
                if v3 > 0:
                    nc.vector.tensor_reduce(out_sb[96:128, g:g + 1],
                                            acc[96:128, :v3],
                                            axis=mybir.AxisListType.X,
                                            op=ALU.add)
                else:
                    nc.vector.memset(out_sb[96:128, g:g + 1], 0.0)

        # ---------------- phase 1 + interleaved GRU ----------------
        ngrp = len(GROUPS)
        jgrp = {j: gi for gi, grp in enumerate(GROUPS) for j in grp}
        done_at = {}                        # group -> #blocks done when ready
        seen = [0] * ngrp
        for q, j in PORDER:
            seen[jgrp[j]] += 1
        cum = 0
        for g in range(ngrp):
            cum += seen[g]
            done_at[g] = cum
        pending = [(g, t) for g in range(ngrp) for t in range(cfg.t)]
        emitted = 0

        for bi, (q, j) in enumerate(PORDER):
            g = jgrp[j]
            lj = j - min(GROUPS[g])
            psumb = ppool.tile([128, cfg.fd], F32, tag="pb")
            oc = opool.tile([128, CPB, 128], BF16, tag="oc")
            nc.vector.tensor_tensor(
                oc[:],
                dstloc_sb[:, bi * CPB:(bi + 1) * CPB]
                .rearrange("p (c o) -> p c o", o=1).to_broadcast([128, CPB, 128]),
                iota128[:].rearrange("p (o s) -> p o s", o=1).to_broadcast([128, CPB, 128]),
                op=ALU.is_equal)
            for jj in range(CPB):
                s = 0 if jj < CHA else 1
                c = bi * CHA + jj if s == 0 else bi * CHB + (jj - CHA)
                gg, slot = divmod(c, CPG)
                gt = gather_tile(s, gg)
                nc.tensor.matmul(
                    psumb[:], lhsT=oc[:, jj, :], rhs=gt[:, slot, :cfg.fd],
                    start=(jj == 0), stop=False)
            nc.tensor.matmul(psumb[:], lhsT=ident[:],
                             rhs=self_sb[:, bi, :cfg.fd],
                             start=False, stop=True)
            xbp = tpool.tile([128, NPAIR, 128], BF16, tag="xb")
            nc.scalar.activation(
                xbp[:, :, 32 * q:32 * q + 32],
                psumb[:].rearrange("p (a b) -> p a b", a=NPAIR),
                AF.Copy, scale=disdst_sb[:, bi:bi + 1])
            for p in range(NPAIR):
                pt = ptpool.tile([128, 128], BF16, tag="pt")
                nc.tensor.transpose(pt[:], xbp[:, p, :], ident[:])
                nc.scalar.copy(
                    xp_g[g][32 * q:32 * q + 32,
                            p * GW[g] + lj * 128:p * GW[g] + (lj + 1) * 128],
                    pt[32 * q:32 * q + 32, :])
            # one pending GRU step per completed block
            if pending and done_at[pending[0][0]] <= bi + 1:
                sg, st = pending.pop(0)
                gru_step(sg, st)
                if st == cfg.t - 1:
                    readout(sg)

        # ---------------- tail: remaining GRU steps ----------------
        rem = {}
        for g, t in pending:
            rem.setdefault(g, []).append(t)
        while rem:
            for g in sorted(rem):
                t = rem[g].pop(0)
                gru_step(g, t, tail=True)
                if t == cfg.t - 1:
                    readout(g)
                if not rem[g]:
                    del rem[g]

        nc.sync.dma_start(out[:], out_sb[:])

    # Spread gathers over the 4 SWDGE queues in final instruction order so
    # the DMASW semaphore-lane rotation maps each lane to one queue.
    cnt = 0
    for f in nc.m.functions:
        for bb in f.blocks:
            for ins in bb.instructions:
                if isinstance(ins, mybir.InstDMAGatherAnt):
                    ins.queue_num = (cnt % 8) % 4
                    cnt += 1

    nc.compile()
    return nc


def _run(cfg=None, trace=False, **inputs):
    if cfg is None:
        cfg = Cfg()
    per_core = partition_graph(cfg, np.asarray(inputs['edge_index']))
    folded = fold_weights(inputs)
    xtab = make_table(cfg, inputs['x'], np.asarray(inputs['edge_index']))
    tab_pad = np.vstack([xtab, np.zeros((1, 256), xtab.dtype)])
    for pc in per_core:
        ids = pc.pop('selfids')                       # [NB, 128]
        st = tab_pad[ids]                             # [NB, 128, 256]
        pc['selftab'] = np.ascontiguousarray(
            st.transpose(1, 0, 2).reshape(128, -1))
    nc = build_nc(cfg, folded['probs'], bias0=not np.any(folded['bias']))
    identw = np.eye(128, dtype=ml_dtypes.bfloat16)
    iotaw = np.tile(np.arange(128, dtype=ml_dtypes.bfloat16), (128, 1))
    shared = {'xtab': xtab, 'wxbd': folded['wxbd'], 'ubd': folded['ubd'],
              'bias': folded['bias'], 'identw': identw, 'iotaw': iotaw}
    in_maps = [{**shared, **pc} for pc in per_core]
    res = run_bass_kernel_spmd(nc, in_maps, core_ids=list(range(cfg.ncores)),
                               trace=trace)
    hsum = np.zeros(cfg.hid, np.float64)
    for r in res.results:
        hsum += (r['out'].astype(np.float64).sum(axis=1)
                 .reshape(4, cfg.hid).sum(0))
    hbar = (hsum / cfg.n).astype(np.float32)[None, :]
    linW = np.asarray(inputs['linW'], np.float32)
    linb = np.asarray(inputs['linb'], np.float32)
    y = np.maximum(hbar @ linW + linb, 0.0).astype(np.float32)
    return y, res


def kernel(**inputs):
    """Grading entry point: full inputs in, full [1, 1] output back."""
    y, _res = _run(cfg=None, trace=False, **inputs)
    return y
